# revision 1
# baseline (speedup 1.0000x reference)
"""AtomAngleProjection distributed Trainium2 kernel (8 NeuronCores).

Reference computation (per batch of B=64 molecules):
  x[b,t] = z[b, i0] + z[b, i1] + z[b, i2]      (3-atom gather-sum per angle)
  h = x @ W1 + b1                               [B*T, 512]
  h = BN(h) with GLOBAL batch stats, * gamma + beta
  out = relu(h) @ W2 + b2                       [B*T, 256]

Strategy: data-parallel over molecules (8 per core). Per core:
  - cast z shard to bf16 in DRAM
  - dma_gather(transpose=True) x3 slots -> X^T tiles [128d, 2, rows] bf16
  - MM1: H^T = W1^T @ X^T (bf16, PSUM f32), +b1 on the PSUM->SBUF copy
    (ACT, fused per-channel sum accumulation); sumsq via DVE scalar_tensor_tensor
  - AllReduce [sum, sumsq] (4KB) -> global mean/var -> s = gamma*rstd,
    t = beta - mean*s
  - relu(s*h+t) on ACT (per-partition scale/bias), MM2 with h'^T chunks as
    stationary weights -> natural-orientation out tiles, +b2, DMA out.
"""
import sys

sys.path.insert(0, "/opt/trn_rl_repo")

import numpy as np

B, N_ATOMS, D_ATOM = 64, 256, 256
T_ANGLES = 2048
D_HID, D_OUT = 512, 256
BN_EPS = 1e-5
N_CORES = 8
B_SH = B // N_CORES                    # molecules per core = 8
R = B_SH * T_ANGLES                    # rows per core = 16384
N_TOTAL = float(B * T_ANGLES)          # BN count = 131072

import os
PHASES = os.environ.get("KERNEL_PHASES", "all")
GMODE = os.environ.get("KERNEL_GMODE", "notr")   # notr | ser
RC = 2048                              # gather / MM1 row chunk
NCH = R // RC                          # 8 chunks
PC = 1024                              # phase-3 row chunk
NPC = R // PC                          # 16 chunks

_CACHE = {}


def build():
    import concourse.bacc as bacc
    import concourse.tile as tile
    import concourse.mybir as mybir

    dt = mybir.dt
    AF = mybir.ActivationFunctionType
    OP = mybir.AluOpType

    from concourse.tile_rust import add_dep_helper

    def raw(i):
        return i.ins if hasattr(i, "ins") and not isinstance(i, mybir.Instruction) else i

    nc = bacc.Bacc(None, target_bir_lowering=False)

    z_ext = nc.declare_dram_parameter("z", [B_SH, N_ATOMS, D_ATOM], dt.float32, isOutput=False)
    tab_ext = nc.declare_dram_parameter("tab", [B_SH, T_ANGLES, 3], dt.int32, isOutput=False)
    w1_ext = nc.declare_dram_parameter("w1", [D_ATOM, D_HID], dt.float32, isOutput=False)
    b1_ext = nc.declare_dram_parameter("b1", [D_HID], dt.float32, isOutput=False)
    g_ext = nc.declare_dram_parameter("gamma", [D_HID], dt.float32, isOutput=False)
    be_ext = nc.declare_dram_parameter("beta", [D_HID], dt.float32, isOutput=False)
    w2_ext = nc.declare_dram_parameter("w2", [D_HID, D_OUT], dt.float32, isOutput=False)
    b2_ext = nc.declare_dram_parameter("b2", [D_OUT], dt.float32, isOutput=False)
    out_ext = nc.declare_dram_parameter("out", [R, D_OUT], dt.float32, isOutput=True)

    with tile.TileContext(nc) as tc:
        with (
            tc.tile_pool(name="dram", bufs=1, space="DRAM") as dram,
            tc.tile_pool(name="const", bufs=1) as cpool,
            tc.tile_pool(name="hbuf", bufs=1) as hpool,
            tc.tile_pool(name="stat", bufs=1) as spool,
            tc.tile_pool(name="ps1", bufs=3, space="PSUM") as ps1,
            tc.tile_pool(name="ps2", bufs=2, space="PSUM") as ps2,
            tc.tile_pool(name="psT", bufs=3, space="PSUM") as psT,
        ):
            # ---------------- constants / weights ----------------
            # W1 as lhsT tiles: w1b[p, kc, m] = W1[kc*128+p, m]  (bf16 cast DMA)
            pre_dmas = []
            w1b = cpool.tile([128, 2, D_HID], dt.bfloat16)
            pre_dmas.append(nc.gpsimd.dma_start(out=w1b[:, :, :], in_=w1_ext.ap().rearrange("(c p) m -> p c m", p=128)))
            # W2 rhs tiles: w2b[p, kc, o] = W2[kc*128+p, o]
            w2b = cpool.tile([128, 4, D_OUT], dt.bfloat16)
            pre_dmas.append(nc.gpsimd.dma_start(out=w2b[:, :, :], in_=w2_ext.ap().rearrange("(c p) m -> p c m", p=128)))
            # channel vectors as [128, 4]: v[p, mc] = vec[mc*128+p]
            b1t = cpool.tile([128, 4], dt.float32)
            pre_dmas.append(nc.sync.dma_start(out=b1t[:, :], in_=b1_ext.ap().rearrange("(m p) -> p m", p=128)))
            gt = cpool.tile([128, 4], dt.float32)
            pre_dmas.append(nc.sync.dma_start(out=gt[:, :], in_=g_ext.ap().rearrange("(m p) -> p m", p=128)))
            bet = cpool.tile([128, 4], dt.float32)
            pre_dmas.append(nc.sync.dma_start(out=bet[:, :], in_=be_ext.ap().rearrange("(m p) -> p m", p=128)))
            # b2 broadcast to all partitions [128, 256]
            b2bc = cpool.tile([128, D_OUT], dt.float32)
            pre_dmas.append(nc.sync.dma_start(out=b2bc[:, :], in_=b2_ext.ap().rearrange("(o d) -> o d", o=1).broadcast_to([128, D_OUT])))

            # persistent index tiles [128, 3, R//16] int16 (wrapped + replicated)
            idx16 = cpool.tile([128, 3, R // 16], dt.int16)
            # identity matrix for PE transposes (notr mode)
            ident = cpool.tile([128, 128], dt.bfloat16)
            colidx = cpool.tile([128, 128], dt.int32)
            pidx = cpool.tile([128, 1], dt.int32)
            nc.gpsimd.iota(colidx[:, :], pattern=[[1, 128]], base=0, channel_multiplier=0)
            nc.gpsimd.iota(pidx[:, :], pattern=[[0, 1]], base=0, channel_multiplier=1)
            colf = cpool.tile([128, 128], dt.float32)
            pidf = cpool.tile([128, 1], dt.float32)
            nc.vector.tensor_copy(colf[:, :], colidx[:, :])
            nc.vector.tensor_copy(pidf[:, :], pidx[:, :])
            nc.vector.tensor_scalar(out=ident[:, :], in0=colf[:, :], scalar1=pidf[:, 0:1],
                                    scalar2=None, op0=OP.is_equal)

            # ---------------- prep (scoped pool, freed early) ----------------
            z16_dram = dram.tile([B_SH * N_ATOMS, D_ATOM], dt.bfloat16)
            with tc.tile_pool(name="prep", bufs=1) as prep:
                # z -> bf16 -> DRAM (gather source), rows = b*256 + atom
                zsb = prep.tile([128, B_SH * N_ATOMS // 128, D_ATOM], dt.bfloat16)
                nc.gpsimd.dma_start(
                    out=zsb[:, :, :],
                    in_=z_ext.ap().rearrange("b a d -> (b a) d").rearrange("(n p) d -> p n d", p=128),
                )
                nc.gpsimd.dma_start(
                    out=z16_dram[:, :].rearrange("(n p) d -> p n d", p=128),
                    in_=zsb[:, :, :],
                )

                # table wrapped load: t32[p, b, c, s] = tab[b, 16c+p, s] (p<16)
                t32 = prep.tile([128, B_SH, T_ANGLES // 16, 3], dt.int32)
                nc.sync.dma_start(
                    out=t32[0:16, :, :, :],
                    in_=tab_ext.ap().rearrange("b (c p) s -> p b c s", p=16),
                )
                # offsets: row index base b*256 for column col = b*128 + c
                offs = prep.tile([128, B_SH, T_ANGLES // 16], dt.int32)
                for bb in range(B_SH):
                    nc.vector.memset(offs[:, bb, :], bb * N_ATOMS)
                idx32 = prep.tile([128, B_SH * T_ANGLES // 16], dt.int32)
                for s in range(3):
                    nc.vector.tensor_tensor(
                        out=idx32[0:16, :],
                        in0=t32[0:16, :, :, s].rearrange("p b c -> p (b c)"),
                        in1=offs[0:16, :, :].rearrange("p b c -> p (b c)"),
                        op=OP.add,
                    )
                    nc.vector.tensor_copy(idx16[0:16, s, :], idx32[0:16, :])
                # replicate to the other 7 16-partition groups (Q7 cores)
                for g in range(1, 8):
                    nc.sync.dma_start(out=idx16[16 * g:16 * (g + 1), :, :], in_=idx16[0:16, :, :])

            # ---------------- persistent H^T: h[p, mc, r] ----------------
            h = hpool.tile([128, 4, R], dt.bfloat16)
            sums_p = spool.tile([128, 4 * NCH * 4], dt.float32)     # per (mc, part)
            sumsq_p = spool.tile([128, 4 * NCH * 4], dt.float32)

            # ---------------- phase 1: gather + MM1 + stats ----------------
            if PHASES in ("all", "12"):
              with (
                tc.tile_pool(name="g", bufs=4) as gpool,
                tc.tile_pool(name="sq", bufs=3) as sqpool,
                tc.tile_pool(name="xt", bufs=3) as xtpool,
              ):
                prev_gather = None
                for ch in range(int(os.environ.get("KERNEL_NCHL", NCH))):
                    gs = []
                    for s in range(3):
                        if GMODE == "ser":
                            gtile = gpool.tile([128, 2, RC], dt.bfloat16, tag="g", name=f"g{s}_{ch}")
                            gi = nc.gpsimd.dma_gather(
                                out_ap=gtile[:, :, :],
                                in_ap=z16_dram[:, :],
                                idxs_ap=idx16[:, s, ch * (RC // 16):(ch + 1) * (RC // 16)],
                                num_idxs=RC,
                                num_idxs_reg=RC,
                                elem_size=D_ATOM,
                                transpose=True,
                                single_packet=False,
                                queue_num=0,
                            )
                            gi = raw(gi)
                            if prev_gather is None:
                                for d in pre_dmas:
                                    add_dep_helper(gi, raw(d), reason="xbar gather after const DMAs")
                            else:
                                add_dep_helper(gi, prev_gather, reason="serialize xbar gathers")
                            prev_gather = gi
                        else:
                            gtile = gpool.tile([128, RC // 128, D_ATOM], dt.bfloat16, tag="g", name=f"g{s}_{ch}")
                            nc.gpsimd.dma_gather(
                                out_ap=gtile[:, :, :],
                                in_ap=z16_dram[:, :],
                                idxs_ap=idx16[:, s, ch * (RC // 16):(ch + 1) * (RC // 16)],
                                num_idxs=RC,
                                num_idxs_reg=RC,
                                elem_size=D_ATOM,
                                transpose=False,
                                single_packet=False,
                                queue_num=0,
                            )
                        gs.append(gtile)
                    # X = g0+g1+g2 (in place into g0)
                    nc.vector.tensor_add(gs[0][:, :, :], gs[0][:, :, :], gs[1][:, :, :])
                    nc.vector.tensor_add(gs[0][:, :, :], gs[0][:, :, :], gs[2][:, :, :])
                    x = gs[0]
                    for rs in range(RC // 512):
                        if GMODE == "ser":
                            xt = x
                            xoff = rs * 512
                        else:
                            # PE-transpose 4 slots x 2 kc -> xt [128, 2, 512]
                            xt = xtpool.tile([128, 2, 512], dt.bfloat16, tag="xt", name=f"xt_{ch}_{rs}")
                            xoff = 0
                            for sl in range(4):
                                for kc in range(2):
                                    ptt = psT.tile([128, 128], dt.bfloat16, tag="psT")
                                    nc.tensor.transpose(ptt[:, :], x[:, rs * 4 + sl, kc * 128:(kc + 1) * 128], ident[:, :])
                                    eng = nc.scalar if (sl + kc) % 2 == 0 else nc.vector
                                    if eng is nc.scalar:
                                        nc.scalar.activation(xt[:, kc, sl * 128:(sl + 1) * 128], ptt[:, :], AF.Copy)
                                    else:
                                        nc.vector.tensor_copy(xt[:, kc, sl * 128:(sl + 1) * 128], ptt[:, :])
                        for mc in range(4):
                            part = (ch * (RC // 512) + rs)
                            pidx2 = mc * (NCH * 4) + part
                            pt = ps1.tile([128, 512], dt.float32, tag="ps1")
                            for kc in range(2):
                                nc.tensor.matmul(
                                    pt[:, :],
                                    w1b[:, kc, mc * 128:(mc + 1) * 128],
                                    xt[:, kc, xoff:xoff + 512],
                                    start=(kc == 0),
                                    stop=(kc == 1),
                                )
                            roff = ch * RC + rs * 512
                            nc.scalar.activation(
                                h[:, mc, roff:roff + 512], pt[:, :], AF.Identity,
                                bias=b1t[:, mc:mc + 1], scale=1.0,
                                accum_out=sums_p[:, pidx2:pidx2 + 1],
                            )
                            hsq = sqpool.tile([128, 512], dt.bfloat16, tag="sq", name=f"sq_{ch}_{rs}_{mc}")
                            nc.vector.scalar_tensor_tensor(
                                out=hsq[:, :], in0=h[:, mc, roff:roff + 512], scalar=1.0,
                                in1=h[:, mc, roff:roff + 512],
                                op0=OP.mult, op1=OP.mult,
                                accum_out=sumsq_p[:, pidx2:pidx2 + 1],
                            )

            # ---------------- phase 2: stats allreduce + affine coeffs ----------------
            sums = spool.tile([128, 4], dt.float32)
            sumsq = spool.tile([128, 4], dt.float32)
            for mc in range(4):
                nc.vector.reduce_sum(out=sums[:, mc:mc + 1], in_=sums_p[:, mc * NCH * 4:(mc + 1) * NCH * 4],
                                     axis=mybir.AxisListType.X)
                nc.vector.reduce_sum(out=sumsq[:, mc:mc + 1], in_=sumsq_p[:, mc * NCH * 4:(mc + 1) * NCH * 4],
                                     axis=mybir.AxisListType.X)
            ar_in = dram.tile([2, D_HID], dt.float32)
            ar_out = dram.tile([2, D_HID], dt.float32, addr_space="Shared")
            nc.sync.dma_start(out=ar_in[0, :].rearrange("(m p) -> p m", p=128), in_=sums[:, :])
            nc.sync.dma_start(out=ar_in[1, :].rearrange("(m p) -> p m", p=128), in_=sumsq[:, :])
            nc.gpsimd.collective_compute(
                "AllReduce", OP.add,
                replica_groups=[list(range(N_CORES))],
                ins=[ar_in[:, :].opt()],
                outs=[ar_out[:, :].opt()],
            )
            sums_g = spool.tile([128, 4], dt.float32)
            sumsq_g = spool.tile([128, 4], dt.float32)
            nc.sync.dma_start(out=sums_g[:, :], in_=ar_out[0, :].rearrange("(m p) -> p m", p=128))
            nc.sync.dma_start(out=sumsq_g[:, :], in_=ar_out[1, :].rearrange("(m p) -> p m", p=128))

            mean = spool.tile([128, 4], dt.float32)
            nc.vector.tensor_scalar(out=mean[:, :], in0=sums_g[:, :], scalar1=1.0 / N_TOTAL,
                                    scalar2=None, op0=OP.mult)
            msq = spool.tile([128, 4], dt.float32)
            nc.vector.tensor_scalar(out=msq[:, :], in0=sumsq_g[:, :], scalar1=1.0 / N_TOTAL,
                                    scalar2=None, op0=OP.mult)
            var = spool.tile([128, 4], dt.float32)
            nc.vector.scalar_tensor_tensor(out=var[:, :], in0=mean[:, :], scalar=-1.0,
                                           in1=mean[:, :], op0=OP.mult, op1=OP.mult)  # -mean^2
            nc.vector.tensor_add(var[:, :], var[:, :], msq[:, :])                      # E[h^2]-mean^2
            epst = spool.tile([128, 1], dt.float32)
            nc.vector.memset(epst[:, :], BN_EPS)
            std = spool.tile([128, 4], dt.float32)
            nc.scalar.activation(std[:, :], var[:, :], AF.Sqrt, bias=epst[:, 0:1], scale=1.0)
            rstd = spool.tile([128, 4], dt.float32)
            nc.vector.reciprocal(rstd[:, :], std[:, :])
            sco = spool.tile([128, 4], dt.float32)
            nc.vector.tensor_mul(sco[:, :], gt[:, :], rstd[:, :])                      # s = gamma*rstd
            tco = spool.tile([128, 4], dt.float32)
            nc.vector.scalar_tensor_tensor(out=tco[:, :], in0=mean[:, :], scalar=-1.0,
                                           in1=sco[:, :], op0=OP.mult, op1=OP.mult)    # -mean*s
            nc.vector.tensor_add(tco[:, :], tco[:, :], bet[:, :])                      # beta - mean*s

            # ---------------- phase 3: relu + MM2 + out ----------------
            if PHASES in ("all", "3"):
              with (
                tc.tile_pool(name="hp", bufs=2) as hppool,
                tc.tile_pool(name="ot", bufs=3) as opool,
              ):
                for pch in range(int(os.environ.get("KERNEL_NPCL", NPC))):
                    hp = hppool.tile([128, 4, PC], dt.bfloat16, tag="hp", name=f"hp_{pch}")
                    for mc in range(4):
                        nc.scalar.activation(
                            hp[:, mc, :], h[:, mc, pch * PC:(pch + 1) * PC], AF.Relu,
                            bias=tco[:, mc:mc + 1], scale=sco[:, mc:mc + 1],
                        )
                    for half in range(2):
                        ot = opool.tile([128, 4, D_OUT], dt.float32, tag="ot", name=f"ot_{pch}_{half}")
                        for sub in range(4):
                            rsub = half * 4 + sub
                            pt2 = ps2.tile([128, D_OUT], dt.float32, tag="ps2")
                            for kc in range(4):
                                nc.tensor.matmul(
                                    pt2[:, :],
                                    hp[:, kc, rsub * 128:(rsub + 1) * 128],
                                    w2b[:, kc, :],
                                    start=(kc == 0),
                                    stop=(kc == 3),
                                )
                            nc.vector.scalar_tensor_tensor(
                                out=ot[:, sub, :], in0=pt2[:, :], scalar=1.0,
                                in1=b2bc[:, :], op0=OP.mult, op1=OP.add,
                            )
                        r0 = pch * PC + half * 512
                        nc.sync.dma_start(
                            out=out_ext[r0:r0 + 512, :].rearrange("(s p) d -> p s d", p=128),
                            in_=ot[:, :, :],
                        )

    if PHASES == "12":
        # still must write the output parameter
        with tile.TileContext(nc) as tc2:
            pass
    nc.compile()
    return nc


def _get_nc():
    if "nc" not in _CACHE:
        _CACHE["nc"] = build()
    return _CACHE["nc"]


def kernel(**inputs) -> np.ndarray:
    from concourse.bass_utils import run_bass_kernel_spmd

    z = np.ascontiguousarray(np.asarray(inputs["z"], dtype=np.float32))
    tab = np.ascontiguousarray(np.asarray(inputs["angel_atom_table"]).astype(np.int32))
    w1 = np.ascontiguousarray(np.asarray(inputs["W1"], dtype=np.float32))
    b1 = np.ascontiguousarray(np.asarray(inputs["b1"], dtype=np.float32))
    gamma = np.ascontiguousarray(np.asarray(inputs["gamma"], dtype=np.float32))
    beta = np.ascontiguousarray(np.asarray(inputs["beta"], dtype=np.float32))
    w2 = np.ascontiguousarray(np.asarray(inputs["W2"], dtype=np.float32))
    b2 = np.ascontiguousarray(np.asarray(inputs["b2"], dtype=np.float32))

    in_maps = []
    for c in range(N_CORES):
        in_maps.append({
            "z": z[c * B_SH:(c + 1) * B_SH],
            "tab": tab[c * B_SH:(c + 1) * B_SH],
            "w1": w1, "b1": b1, "gamma": gamma, "beta": beta, "w2": w2, "b2": b2,
        })

    import time as _t
    print(f"[kernel] building...", flush=True)
    _t0 = _t.time()
    nc = _get_nc()
    print(f"[kernel] built in {_t.time()-_t0:.0f}s; running...", flush=True)
    _t0 = _t.time()
    res = run_bass_kernel_spmd(nc, in_maps, core_ids=list(range(N_CORES)))
    print(f"[kernel] ran in {_t.time()-_t0:.0f}s", flush=True)
    out = np.concatenate([res.results[c]["out"] for c in range(N_CORES)], axis=0)
    return out.astype(np.float32)


if __name__ == "__main__":
    # quick self-exercise with random inputs (shapes only)
    rng = np.random.default_rng(0)
    ins = {
        "z": rng.standard_normal((B, N_ATOMS, D_ATOM), dtype=np.float32),
        "angel_atom_table": rng.integers(0, N_ATOMS, (B, T_ANGLES, 3)).astype(np.int32),
        "W1": rng.standard_normal((D_ATOM, D_HID), dtype=np.float32) / 16.0,
        "b1": rng.standard_normal(D_HID).astype(np.float32) * 0.01,
        "gamma": np.ones(D_HID, dtype=np.float32),
        "beta": np.zeros(D_HID, dtype=np.float32),
        "W2": rng.standard_normal((D_HID, D_OUT), dtype=np.float32) / 22.0,
        "b2": rng.standard_normal(D_OUT).astype(np.float32) * 0.01,
    }
    out = kernel(**ins)
    print("kernel out:", out.shape, out.dtype, float(np.abs(out).mean()))



# revision 6
# speedup vs baseline: 1.7010x; 1.7010x over previous
"""AtomAngleProjection distributed Trainium2 kernel (8 NeuronCores).

Reference computation (per batch of B=64 molecules):
  x[b,t] = z[b, i0] + z[b, i1] + z[b, i2]      (3-atom gather-sum per angle)
  h = x @ W1 + b1                               [B*T, 512]
  h = BN(h) with GLOBAL batch stats, * gamma + beta
  out = relu(h) @ W2 + b2                       [B*T, 256]

Strategy: data-parallel over molecules (8 per core). The gather-sum is
reformulated as a dense matmul: with A^T[a, t] = sum_s (idx[t,s] == a)
(a one-hot-count matrix built on the DVE), per molecule

  X = A @ z_b            so    H~^T = (U^T A^T)  with U = z_b @ W1.

b1 is dropped entirely (it cancels inside BatchNorm: mean shifts by b1
so h - mean is unchanged). Per core:
  - z -> bf16 in DRAM, XBAR DMA-transpose -> z^T tiles (lhsT for U)
  - per molecule: broadcast idx rows -> [128, 2048], build A^T via
    is_equal/add on DVE, U = z^T.T @ W1 (PE), H~^T = U^T @ A^T (PE),
    PSUM->SBUF copies accumulate per-channel sums; sumsq via ACT Square
  - AllReduce [sum, sumsq] (4KB) -> global mean/var -> s = gamma*rstd,
    t = beta - mean*s
  - relu(s*h~+t) split ACT/DVE, MM2 with W2 stationary -> out^T in PSUM,
    +b2 on the PSUM->SBUF copy (per-partition), DMA out^T -> DRAM.
Host un-transposes out^T per core.
"""
import sys

sys.path.insert(0, "/opt/trn_rl_repo")

import numpy as np

B, N_ATOMS, D_ATOM = 64, 256, 256
T_ANGLES = 2048
D_HID, D_OUT = 512, 256
BN_EPS = 1e-5
N_CORES = 8
B_SH = B // N_CORES                    # molecules per core = 8
R = B_SH * T_ANGLES                    # rows per core = 16384
N_TOTAL = float(B * T_ANGLES)          # BN count = 131072

_CACHE = {}


def build():
    import concourse.bacc as bacc
    import concourse.tile as tile
    import concourse.mybir as mybir

    dt = mybir.dt
    AF = mybir.ActivationFunctionType
    OP = mybir.AluOpType

    nc = bacc.Bacc(None, target_bir_lowering=False)

    z_ext = nc.declare_dram_parameter("z", [B_SH, N_ATOMS, D_ATOM], dt.float32, isOutput=False)
    tab_ext = nc.declare_dram_parameter("tab", [B_SH, T_ANGLES, 3], dt.int32, isOutput=False)
    w1_ext = nc.declare_dram_parameter("w1", [D_ATOM, D_HID], dt.float32, isOutput=False)
    g_ext = nc.declare_dram_parameter("gamma", [D_HID], dt.float32, isOutput=False)
    be_ext = nc.declare_dram_parameter("beta", [D_HID], dt.float32, isOutput=False)
    w2_ext = nc.declare_dram_parameter("w2", [D_HID, D_OUT], dt.float32, isOutput=False)
    b2_ext = nc.declare_dram_parameter("b2", [D_OUT], dt.float32, isOutput=False)
    # output is written transposed: outT[o, r] = out[r, o]
    out_ext = nc.declare_dram_parameter("outT", [D_OUT, R], dt.float32, isOutput=True)

    with tile.TileContext(nc) as tc:
        with (
            tc.tile_pool(name="dram", bufs=1, space="DRAM") as dram,
            tc.tile_pool(name="const", bufs=1) as cpool,
            tc.tile_pool(name="hbuf", bufs=1) as hpool,
            tc.tile_pool(name="stat", bufs=1) as spool,
        ):
            # ---------------- constants / weights ----------------
            # W1 rhs tiles: w1r[p, dc, m] = W1[dc*128+p, m]  (bf16 cast DMA)
            w1r = cpool.tile([128, 2, D_HID], dt.bfloat16)
            nc.gpsimd.dma_start(out=w1r[:, :, :], in_=w1_ext.ap().rearrange("(c p) m -> p c m", p=128))
            # W2 lhsT tiles: w2r[p, kc, o] = W2[kc*128+p, o]
            w2r = cpool.tile([128, 4, D_OUT], dt.bfloat16)
            nc.gpsimd.dma_start(out=w2r[:, :, :], in_=w2_ext.ap().rearrange("(c p) m -> p c m", p=128))
            # channel vectors as [128, nc]: v[p, c] = vec[c*128+p]
            gt = cpool.tile([128, 4], dt.float32)
            nc.sync.dma_start(out=gt[:, :], in_=g_ext.ap().rearrange("(m p) -> p m", p=128))
            bet = cpool.tile([128, 4], dt.float32)
            nc.sync.dma_start(out=bet[:, :], in_=be_ext.ap().rearrange("(m p) -> p m", p=128))
            b2t = cpool.tile([128, 2], dt.float32)
            nc.sync.dma_start(out=b2t[:, :], in_=b2_ext.ap().rearrange("(o p) -> p o", p=128))

            # per-partition atom values for the one-hot compare: aval[p, ac] = ac*128 + p
            pidx = cpool.tile([128, 1], dt.int32)
            nc.gpsimd.iota(pidx[:, :], pattern=[[0, 1]], base=0, channel_multiplier=1)
            pidf = cpool.tile([128, 1], dt.float32)
            nc.vector.tensor_copy(pidf[:, :], pidx[:, :])
            aval = cpool.tile([128, 2], dt.float32)
            nc.vector.tensor_scalar(out=aval[:, 0:1], in0=pidf[:, :], scalar1=0.0,
                                    scalar2=None, op0=OP.add)
            nc.vector.tensor_scalar(out=aval[:, 1:2], in0=pidf[:, :], scalar1=128.0,
                                    scalar2=None, op0=OP.add)

            # ---------------- prep in DRAM ----------------
            z16_dram = dram.tile([B_SH * N_ATOMS, D_ATOM], dt.bfloat16)
            tabd = dram.tile([B_SH * 3, T_ANGLES], dt.bfloat16)
            with tc.tile_pool(name="prep", bufs=1) as prep:
                # z -> bf16 -> DRAM (rows = b*256 + atom)
                zsb = prep.tile([128, B_SH * N_ATOMS // 128, D_ATOM], dt.bfloat16)
                nc.gpsimd.dma_start(
                    out=zsb[:, :, :],
                    in_=z_ext.ap().rearrange("b a d -> (b a) d").rearrange("(n p) d -> p n d", p=128),
                )
                nc.gpsimd.dma_start(
                    out=z16_dram[:, :].rearrange("(n p) d -> p n d", p=128),
                    in_=zsb[:, :, :],
                )
                # table: strided load [b, s, t] int32 -> bf16 -> DRAM rows of 4KB
                t32 = prep.tile([B_SH, 3, T_ANGLES], dt.int32)
                for s in range(3):
                    nc.sync.dma_start(out=t32[:, s, :],
                                      in_=tab_ext.ap().rearrange("b t s -> b s t")[:, s, :])
                t16 = prep.tile([B_SH, 3, T_ANGLES], dt.bfloat16)
                nc.vector.tensor_copy(t16[:, :, :], t32[:, :, :])
                nc.sync.dma_start(out=tabd[:, :].rearrange("(b s) t -> b s t", b=B_SH), in_=t16[:, :, :])

            # z^T via XBAR DMA transpose: zTt[p, dc, r] = z16[r, dc*128+p], r = b*256+a
            zTt = cpool.tile([128, 2, B_SH * N_ATOMS], dt.bfloat16)
            for dc in range(2):
                nc.sync.dma_start_transpose(out=zTt[:, dc, :], in_=z16_dram[:, dc * 128:(dc + 1) * 128])

            # ---------------- persistent H~^T: h[p, mc, r] ----------------
            h = hpool.tile([128, 4, R], dt.bfloat16)
            sums_p = spool.tile([128, 4, 4 * B_SH], dt.float32)     # per (mc, (b, tq))
            sumsq_p = spool.tile([128, 4, B_SH], dt.float32)        # per (mc, b)

            # ---------------- phase 1: one-hot + U + H~^T + stats ----------------
            with (
                tc.tile_pool(name="idxp", bufs=2) as idxp,
                tc.tile_pool(name="atp", bufs=2) as atp,
                tc.tile_pool(name="utp", bufs=2) as utp,
                tc.tile_pool(name="sqp", bufs=2) as sqp,
                tc.tile_pool(name="psU", bufs=2, space="PSUM") as psUp,
                tc.tile_pool(name="psH", bufs=4, space="PSUM") as psHp,
            ):
                for b in range(B_SH):
                    # broadcast idx rows to all partitions
                    idxrep = idxp.tile([128, 3, T_ANGLES], dt.bfloat16, tag="idx", name=f"idx_{b}")
                    for s in range(3):
                        nc.sync.dma_start(
                            out=idxrep[:, s, :],
                            in_=tabd[b * 3 + s:b * 3 + s + 1, :].broadcast_to([128, T_ANGLES]),
                        )
                    # one-hot count matrix A^T[a, t] (a = ac*128 + p)
                    at = atp.tile([128, 2, T_ANGLES], dt.bfloat16, tag="at", name=f"at_{b}")
                    for ac in range(2):
                        nc.vector.tensor_scalar(
                            out=at[:, ac, :], in0=idxrep[:, 0, :],
                            scalar1=aval[:, ac:ac + 1], scalar2=None, op0=OP.is_equal,
                        )
                        for s in (1, 2):
                            nc.vector.scalar_tensor_tensor(
                                out=at[:, ac, :], in0=idxrep[:, s, :],
                                scalar=aval[:, ac:ac + 1], in1=at[:, ac, :],
                                op0=OP.is_equal, op1=OP.add,
                            )
                    # U = z_b @ W1  ->  Ut[p, ac, m] (bf16)
                    ut = utp.tile([128, 2, D_HID], dt.bfloat16, tag="ut", name=f"ut_{b}")
                    for ac in range(2):
                        pu = psUp.tile([128, D_HID], dt.float32, tag="psU")
                        for dc in range(2):
                            nc.tensor.matmul(
                                pu[:, :],
                                zTt[:, dc, b * N_ATOMS + ac * 128: b * N_ATOMS + (ac + 1) * 128],
                                w1r[:, dc, :],
                                start=(dc == 0), stop=(dc == 1),
                            )
                        nc.vector.tensor_scalar(out=ut[:, ac, :], in0=pu[:, :], scalar1=1.0,
                                                scalar2=None, op0=OP.mult)
                    # H~^T[m, t] = U^T A^T ; PSUM->SBUF copy accumulates sums
                    for mc in range(4):
                        for tq in range(4):
                            ph = psHp.tile([128, 512], dt.float32, tag="psH")
                            for ac in range(2):
                                nc.tensor.matmul(
                                    ph[:, :],
                                    ut[:, ac, mc * 128:(mc + 1) * 128],
                                    at[:, ac, tq * 512:(tq + 1) * 512],
                                    start=(ac == 0), stop=(ac == 1),
                                )
                            roff = b * T_ANGLES + tq * 512
                            scol = sums_p[:, mc, b * 4 + tq: b * 4 + tq + 1]
                            if (mc + tq) % 3 == 0:   # ~1/3 of copies on ACT
                                nc.scalar.activation(
                                    h[:, mc, roff:roff + 512], ph[:, :], AF.Copy,
                                    bias=0.0, scale=1.0, accum_out=scol,
                                )
                            else:
                                nc.vector.tensor_scalar(
                                    out=h[:, mc, roff:roff + 512], in0=ph[:, :],
                                    scalar1=1.0, scalar2=0.0, op0=OP.mult, op1=OP.add,
                                    accum_out=scol,
                                )
                    # sumsq via ACT Square (scratch output, accumulator is the point)
                    for mc in range(4):
                        sq = sqp.tile([128, T_ANGLES], dt.bfloat16, tag="sq", name=f"sq_{b}_{mc}")
                        nc.scalar.activation(
                            sq[:, :], h[:, mc, b * T_ANGLES:(b + 1) * T_ANGLES], AF.Square,
                            accum_out=sumsq_p[:, mc, b:b + 1],
                        )

            # ---------------- phase 2: stats allreduce + affine coeffs ----------------
            sums = spool.tile([128, 4], dt.float32)
            sumsq = spool.tile([128, 4], dt.float32)
            for mc in range(4):
                nc.vector.reduce_sum(out=sums[:, mc:mc + 1], in_=sums_p[:, mc, :],
                                     axis=mybir.AxisListType.X)
                nc.vector.reduce_sum(out=sumsq[:, mc:mc + 1], in_=sumsq_p[:, mc, :],
                                     axis=mybir.AxisListType.X)
            ar_in = dram.tile([2, D_HID], dt.float32)
            ar_out = dram.tile([2, D_HID], dt.float32, addr_space="Shared")
            nc.sync.dma_start(out=ar_in[0, :].rearrange("(m p) -> p m", p=128), in_=sums[:, :])
            nc.sync.dma_start(out=ar_in[1, :].rearrange("(m p) -> p m", p=128), in_=sumsq[:, :])
            nc.gpsimd.collective_compute(
                "AllReduce", OP.add,
                replica_groups=[list(range(N_CORES))],
                ins=[ar_in[:, :].opt()],
                outs=[ar_out[:, :].opt()],
            )
            sums_g = spool.tile([128, 4], dt.float32)
            sumsq_g = spool.tile([128, 4], dt.float32)
            nc.sync.dma_start(out=sums_g[:, :], in_=ar_out[0, :].rearrange("(m p) -> p m", p=128))
            nc.sync.dma_start(out=sumsq_g[:, :], in_=ar_out[1, :].rearrange("(m p) -> p m", p=128))

            mean = spool.tile([128, 4], dt.float32)
            nc.vector.tensor_scalar(out=mean[:, :], in0=sums_g[:, :], scalar1=1.0 / N_TOTAL,
                                    scalar2=None, op0=OP.mult)
            msq = spool.tile([128, 4], dt.float32)
            nc.vector.tensor_scalar(out=msq[:, :], in0=sumsq_g[:, :], scalar1=1.0 / N_TOTAL,
                                    scalar2=None, op0=OP.mult)
            var = spool.tile([128, 4], dt.float32)
            nc.vector.scalar_tensor_tensor(out=var[:, :], in0=mean[:, :], scalar=-1.0,
                                           in1=mean[:, :], op0=OP.mult, op1=OP.mult)  # -mean^2
            nc.vector.tensor_add(var[:, :], var[:, :], msq[:, :])                      # E[h^2]-mean^2
            epst = spool.tile([128, 1], dt.float32)
            nc.vector.memset(epst[:, :], BN_EPS)
            std = spool.tile([128, 4], dt.float32)
            nc.scalar.activation(std[:, :], var[:, :], AF.Sqrt, bias=epst[:, 0:1], scale=1.0)
            rstd = spool.tile([128, 4], dt.float32)
            nc.vector.reciprocal(rstd[:, :], std[:, :])
            sco = spool.tile([128, 4], dt.float32)
            nc.vector.tensor_mul(sco[:, :], gt[:, :], rstd[:, :])                      # s = gamma*rstd
            tco = spool.tile([128, 4], dt.float32)
            nc.vector.scalar_tensor_tensor(out=tco[:, :], in0=mean[:, :], scalar=-1.0,
                                           in1=sco[:, :], op0=OP.mult, op1=OP.mult)    # -mean*s
            nc.vector.tensor_add(tco[:, :], tco[:, :], bet[:, :])                      # beta - mean*s

            # ---------------- phase 3: relu + MM2 (W2 stationary) + outT ----------------
            PC = 1024
            NPC = R // PC                  # 16 chunks
            with (
                tc.tile_pool(name="hp", bufs=2) as hppool,
                tc.tile_pool(name="rt", bufs=2) as rtpool,
                tc.tile_pool(name="ot", bufs=2) as opool,
                tc.tile_pool(name="psO", bufs=4, space="PSUM") as psOp,
            ):
                for pch in range(NPC):
                    hp = hppool.tile([128, 4, PC], dt.bfloat16, tag="hp", name=f"hp_{pch}")
                    for mc in range(4):
                        if mc < 2:
                            nc.scalar.activation(
                                hp[:, mc, :], h[:, mc, pch * PC:(pch + 1) * PC], AF.Relu,
                                bias=tco[:, mc:mc + 1], scale=sco[:, mc:mc + 1],
                            )
                        else:
                            rtmp = rtpool.tile([128, PC], dt.bfloat16, tag="rt", name=f"rt_{pch}_{mc}")
                            nc.vector.tensor_scalar(
                                out=rtmp[:, :], in0=h[:, mc, pch * PC:(pch + 1) * PC],
                                scalar1=sco[:, mc:mc + 1], scalar2=tco[:, mc:mc + 1],
                                op0=OP.mult, op1=OP.add,
                            )
                            nc.vector.tensor_scalar(
                                out=hp[:, mc, :], in0=rtmp[:, :],
                                scalar1=0.0, scalar2=None, op0=OP.max,
                            )
                    ot = opool.tile([128, 2, PC], dt.float32, tag="ot", name=f"ot_{pch}")
                    for tq in range(2):
                        for oc in range(2):
                            po = psOp.tile([128, 512], dt.float32, tag="psO")
                            for kc in range(4):
                                nc.tensor.matmul(
                                    po[:, :],
                                    w2r[:, kc, oc * 128:(oc + 1) * 128],
                                    hp[:, kc, tq * 512:(tq + 1) * 512],
                                    start=(kc == 0), stop=(kc == 3),
                                )
                            nc.vector.tensor_scalar(
                                out=ot[:, oc, tq * 512:(tq + 1) * 512], in0=po[:, :],
                                scalar1=b2t[:, oc:oc + 1], scalar2=None, op0=OP.add,
                            )
                    nc.sync.dma_start(
                        out=out_ext.ap().rearrange("(oc p) t -> p oc t", p=128)[:, :, pch * PC:(pch + 1) * PC],
                        in_=ot[:, :, :],
                    )

    nc.compile()
    return nc


def _get_nc():
    if "nc" not in _CACHE:
        _CACHE["nc"] = build()
    return _CACHE["nc"]


def make_in_maps(inputs):
    z = np.ascontiguousarray(np.asarray(inputs["z"], dtype=np.float32))
    tab = np.ascontiguousarray(np.asarray(inputs["angel_atom_table"]).astype(np.int32))
    w1 = np.ascontiguousarray(np.asarray(inputs["W1"], dtype=np.float32))
    gamma = np.ascontiguousarray(np.asarray(inputs["gamma"], dtype=np.float32))
    beta = np.ascontiguousarray(np.asarray(inputs["beta"], dtype=np.float32))
    w2 = np.ascontiguousarray(np.asarray(inputs["W2"], dtype=np.float32))
    b2 = np.ascontiguousarray(np.asarray(inputs["b2"], dtype=np.float32))
    in_maps = []
    for c in range(N_CORES):
        in_maps.append({
            "z": z[c * B_SH:(c + 1) * B_SH],
            "tab": tab[c * B_SH:(c + 1) * B_SH],
            "w1": w1, "gamma": gamma, "beta": beta, "w2": w2, "b2": b2,
        })
    return in_maps


def assemble_out(res):
    # each core returns outT [256, 16384]; un-transpose and stack
    return np.concatenate(
        [np.asarray(res.results[c]["outT"]).T for c in range(N_CORES)], axis=0
    ).astype(np.float32)


def kernel(**inputs) -> np.ndarray:
    from concourse.bass_utils import run_bass_kernel_spmd

    in_maps = make_in_maps(inputs)
    nc = _get_nc()
    res = run_bass_kernel_spmd(nc, in_maps, core_ids=list(range(N_CORES)))
    return assemble_out(res)


if __name__ == "__main__":
    rng = np.random.default_rng(0)
    ins = {
        "z": rng.standard_normal((B, N_ATOMS, D_ATOM), dtype=np.float32),
        "angel_atom_table": rng.integers(0, N_ATOMS, (B, T_ANGLES, 3)).astype(np.int32),
        "W1": rng.standard_normal((D_ATOM, D_HID), dtype=np.float32) / 16.0,
        "b1": rng.standard_normal(D_HID).astype(np.float32) * 0.01,
        "gamma": np.ones(D_HID, dtype=np.float32),
        "beta": np.zeros(D_HID, dtype=np.float32),
        "W2": rng.standard_normal((D_HID, D_OUT), dtype=np.float32) / 22.0,
        "b2": rng.standard_normal(D_OUT).astype(np.float32) * 0.01,
    }
    out = kernel(**ins)
    print("kernel out:", out.shape, out.dtype, float(np.abs(out).mean()))
